# revision 18
# baseline (speedup 1.0000x reference)
"""Trainium2 Bass kernel: dual cross-attention block (nn_CA_36670430773307).

Full-input contract: kernel(**inputs) takes the complete unsharded tensors and
returns the complete (4, 4096, 512) output.

Sharding: 8 cores = batch(4) x direction(2). Each core computes one full
cross-attention direction (t->i or i->t) for one batch element:
    xq_ln = LN(xq);  xkv_ln = LN(xkv)
    q = xq_ln @ w_q * 0.125          (0.125 and gamma folded into w_q on host)
    k = xkv_ln @ w_k ; v = xkv_ln @ w_v
    out = softmax(q @ k^T) @ v @ w_out          (per head, 8 heads)
gamma is folded into the projection weights on the host; beta contributes
beta@W, added as a per-partition bias for feature-major q/k and via a K=1
ones-matmul for token-major v.

Engine assignment keeps the ACT (scalar) engine exclusively on softmax Exp —
the kernel's hard floor (33.5M exp elements ~ 300us at 1 elem/cycle/lane):
  - LN stats: bn_stats/bn_aggr on DVE; batched sqrt(var+eps) on ACT ([P,4]
    per 512-token group) and one batched DVE reciprocal
  - LN apply: tensor_scalar on the GPSIMD engine (SBUF->SBUF)
  - transpose to d-major x^T: dma_start_transpose (XBAR, SBUF->SBUF, bit
    exact) into token-tile-major xT2 [P, NT, DC, 128] whose per-partition
    destination blocks are contiguous -- no PE transposes, no PSUM traffic
  - projections per 512-token group, PSUM->SBUF copies on DVE (bias folded)
  - attention per (head-pair, 2-query-chunk pass): sim^T = k_h @ q_h^T (the
    two heads of a pair live on partition halves so their K=64 matmuls run
    concurrently via PE row groups), Exp over [128,1024] PSUM -> bf16, AV
    matmuls accumulate [65,512] with a ones column producing the softmax
    denominator in row 64
  - normalization: one fp32 copy [65,512] frees the AV PSUM bank (so the
    next pass's AV matmuls never stall and the PE stays warm/un-throttled);
    denominator rows are DMA-gathered into [4,512], ONE batched DVE
    reciprocal per pass, DMA broadcast back, multiply on GPSIMD
  - output projection: head pairs stacked on 128 partitions -> K=128
    matmuls accumulated in one PSUM bank -> bf16 out (host upcasts)
No max-subtraction in softmax: logits are ~N(0, 0.2), exp is safe.

Walrus allows 1 sync wait per instruction; _legalize_waits splits extras
onto same-engine EventSemaphore instructions. scalar_tensor_tensor (opcode
0x9d) is rejected by this NRT at NEFF load -- avoided.
"""

import numpy as np
import ml_dtypes

import concourse.bass as bass
import concourse.mybir as mybir
import concourse.tile as tile
from concourse.bass_utils import run_bass_kernel_spmd

N = 2048            # tokens per stream
D = 512             # model dim
H = 8               # heads
HD = 64             # head dim
P = 128             # SBUF partitions
NT = N // P         # 16 token tiles
DC = D // P         # 4 model-dim chunks
IC = 512            # attention query chunk == PSUM bank free size fp32
NIC = N // IC       # 4 query chunks
LN_EPS = 1e-5

F32 = mybir.dt.float32
BF16 = mybir.dt.bfloat16
ALU = mybir.AluOpType
ACTF = mybir.ActivationFunctionType

LAST_EXEC_NS = None
_NC_CACHE = None


def _legalize_waits(js):
    """Walrus encodes ONE sync wait per instruction (the ISA EVENTS slot).
    Tile's wait assignment can attach several. Split the surplus onto
    preceding same-engine EventSemaphore instructions."""
    n_split = 0
    for f in js["functions"]:
        for b in f["blocks"]:
            out = []
            for ins in b["instructions"]:
                si = ins.get("sync_info") or {}
                ow = si.get("on_wait") or []
                if len(ow) > 1:
                    for k, w in enumerate(ow[:-1]):
                        out.append({
                            "debug": ins.get("debug"),
                            "engine": ins["engine"],
                            "ins": [], "outs": [],
                            "name": f"{ins['name']}_w{k}",
                            "opcode": "EventSemaphore",
                            "sync_info": {"on_update": [], "on_wait": [w]},
                        })
                        n_split += 1
                    si = dict(si)
                    si["on_wait"] = [ow[-1]]
                    ins = dict(ins)
                    ins["sync_info"] = si
                out.append(ins)
            b["instructions"] = out
    return n_split


def _build_program():
    nc = bass.Bass()

    xq = nc.declare_dram_parameter("xq", [N, D], BF16, isOutput=False)
    xkv = nc.declare_dram_parameter("xkv", [N, D], BF16, isOutput=False)
    wq = nc.declare_dram_parameter("wq", [D, D], BF16, isOutput=False)
    wkv = nc.declare_dram_parameter("wkv", [D, 2 * D], BF16, isOutput=False)
    wout = nc.declare_dram_parameter("wout", [D, D], BF16, isOutput=False)
    qb = nc.declare_dram_parameter("qb", [D], F32, isOutput=False)
    kb = nc.declare_dram_parameter("kb", [D], F32, isOutput=False)
    vb = nc.declare_dram_parameter("vb", [D], BF16, isOutput=False)
    outs = [
        nc.declare_dram_parameter(f"out{g}", [P, 4, D], BF16, isOutput=True)
        for g in range(NT // 4)
    ]

    with tile.TileContext(nc) as tc:
        _body(tc, xq, xkv, wq, wkv, wout, qb, kb, vb, outs)

    import json
    js = json.loads(nc.to_json_bytes())
    _legalize_waits(js)
    legalized = json.dumps(js).encode()
    nc.to_json_bytes = lambda: legalized
    return nc


def _ln_group(tc, lns, lnxs, src_big, g, eps_sb):
    """LN for 4 token tiles of one stream. Stats on DVE, batched sqrt on
    ACT, batched reciprocal on DVE, apply on GPSIMD. Returns bf16 tiles."""
    nc = tc.nc
    mvg = lns.tile([P, 4, 2], F32, tag="mv", name="mvg")
    for kk in range(4):
        it = g * 4 + kk
        st = lns.tile([P, 6], F32, tag="st", name="st")
        nc.vector.bn_stats(out=st, in_=src_big[:, it, :])
        nc.vector.bn_aggr(out=mvg[:, kk, :], in_=st)
    sdg = lns.tile([P, 4], F32, tag="sd", name="sdg")
    nc.scalar.activation(out=sdg, in_=mvg[:, :, 1], func=ACTF.Sqrt,
                         bias=eps_sb)
    invg = lns.tile([P, 4], F32, tag="inv", name="invg")
    nc.vector.reciprocal(out=invg, in_=sdg)
    nmig = lns.tile([P, 4], F32, tag="nmi", name="nmig")
    nc.vector.tensor_tensor(out=nmig, in0=mvg[:, :, 0], in1=invg,
                            op=ALU.mult)
    nc.vector.tensor_scalar(out=nmig, in0=nmig, scalar1=-1.0,
                            scalar2=None, op0=ALU.mult)
    xss = []
    for kk in range(4):
        it = g * 4 + kk
        xs = lnxs.tile([P, D], BF16, name="xs")
        nc.gpsimd.tensor_scalar(
            out=xs, in0=src_big[:, it, :],
            scalar1=invg[:, kk:kk + 1], scalar2=nmig[:, kk:kk + 1],
            op0=ALU.mult, op1=ALU.add,
        )
        xss.append(xs)
    return xss


def _body(tc, xq, xkv, wq, wkv, wout, qb, kb, vb, outs):
    nc = tc.nc

    with (
        tc.tile_pool(name="persist", bufs=1) as pers,
        tc.tile_pool(name="lns", bufs=8) as lns,
        tc.tile_pool(name="lnxs", bufs=10) as lnxs,
        tc.tile_pool(name="lnx", bufs=2) as lnx,
        tc.tile_pool(name="expp", bufs=2) as expp,
        tc.tile_pool(name="smallp", bufs=3) as smallp,
        tc.tile_pool(name="avsbp", bufs=8) as avsbp,
        tc.tile_pool(name="bigp", bufs=2) as bigp,
        tc.tile_pool(name="dramp", bufs=8, space="DRAM") as dramp,
        tc.tile_pool(name="ps_pool", bufs=2, space="PSUM") as ps_pool,
    ):
        # ---- persistent tiles ----
        eps_sb = pers.tile([P, 1], F32, name="eps_sb")
        nc.vector.memset(eps_sb, LN_EPS)
        ones1p = pers.tile([1, P], BF16, name="ones1p")
        nc.vector.memset(ones1p, 1.0)

        # weights + biases on the HW DGE (sync) ring; x loads on the gpsimd
        # SW ring (8 DMAs, one per token group) so SW lanes never wrap
        wq_sb = pers.tile([P, DC, D], BF16, name="wq_sb")
        nc.sync.dma_start(out=wq_sb, in_=wq.rearrange("(c p) f -> p c f", p=P))
        wkv_sb = pers.tile([P, DC, 2 * D], BF16, name="wkv_sb")
        nc.sync.dma_start(out=wkv_sb, in_=wkv.rearrange("(c p) f -> p c f", p=P))
        # w_out rows stacked per head PAIR: [128, H//2, D]
        wout_sb = pers.tile([P, H // 2, D], BF16, name="wout_sb")
        nc.sync.dma_start(out=wout_sb, in_=wout.rearrange("(c p) f -> p c f", p=P))
        qb_sb = pers.tile([P, DC], F32, name="qb_sb")
        nc.sync.dma_start(out=qb_sb, in_=qb.rearrange("(c p) -> p c", p=P))
        kb_sb = pers.tile([P, DC], F32, name="kb_sb")
        nc.sync.dma_start(out=kb_sb, in_=kb.rearrange("(c p) -> p c", p=P))
        vb_sb = pers.tile([1, D], BF16, name="vb_sb")
        nc.sync.dma_start(out=vb_sb, in_=vb.rearrange("(o d) -> o d", o=1))

        # x^T in token-tile-major layout: xT2[p, it, c, f] = x[it*128+f,
        # c*128+p]; each dma_start_transpose writes one contiguous
        # [128, DC*128] per-partition block
        xqT = bigp.tile([P, NT, DC, P], BF16, tag="big", name="xqT")
        xkvT = bigp.tile([P, NT, DC, P], BF16, tag="big", name="xkvT")
        qT = pers.tile([P, DC, N], BF16, name="qT")   # q^T feature-major
        kT = pers.tile([P, DC, N], BF16, name="kT")   # head h: chunk h//2,
        #                                               partitions (h%2)*64
        vaug = pers.tile([P, NT, H, HD + 1], BF16, name="vaug")
        nc.vector.memset(vaug[:, :, :, HD:HD + 1], 1.0)
        # normalized attention output, head pairs stacked on partitions:
        # head 2*hp+side lives at partitions side*64..(side+1)*64
        aout = pers.tile([P, H // 2, N], BF16, name="aout")

        xqbig = lnx.tile([P, NT, D], BF16, tag="xbig", name="xqbig")
        xkvbig = lnx.tile([P, NT, D], BF16, tag="xbig", name="xkvbig")
        xq_r = xq.rearrange("(t p) d -> p t d", p=P)
        xkv_r = xkv.rearrange("(t p) d -> p t d", p=P)

        # ---- phases A+B pipelined per 512-token group ----
        for g in range(NT // 4):
            nc.gpsimd.dma_start(out=xqbig[:, g * 4:(g + 1) * 4, :],
                                in_=xq_r[:, g * 4:(g + 1) * 4, :])
            nc.gpsimd.dma_start(out=xkvbig[:, g * 4:(g + 1) * 4, :],
                                in_=xkv_r[:, g * 4:(g + 1) * 4, :])
            for xbig, xT in ((xqbig, xqT), (xkvbig, xkvT)):
                xss = _ln_group(tc, lns, lnxs, xbig, g, eps_sb)
                for kk in range(4):
                    it = g * 4 + kk
                    nc.sync.dma_start_transpose(xT[:, it, :, :], xss[kk])

            # projections for this token group: k and v first (attention
            # consumes them per j-tile, so the tail group must land early),
            # q last. Copies for groups 0-2 ride the idle ACT engine; group
            # 3's go on DVE so the exp queue never waits on them.
            for m in range(DC):
                ps = ps_pool.tile([P, IC], F32, tag="av", bufs=4, name="psk")
                for k in range(DC):
                    nc.tensor.matmul(
                        ps,
                        lhsT=wkv_sb[:, k, m * P:(m + 1) * P],
                        rhs=xkvT[:, g * 4:(g + 1) * 4, k, :],
                        start=(k == 0), stop=(k == DC - 1),
                    )
                if g < 3:
                    nc.scalar.activation(
                        out=kT[:, m, g * IC:(g + 1) * IC], in_=ps,
                        func=ACTF.Identity, bias=kb_sb[:, m:m + 1],
                    )
                else:
                    nc.vector.tensor_scalar(
                        out=kT[:, m, g * IC:(g + 1) * IC], in0=ps,
                        scalar1=kb_sb[:, m:m + 1], scalar2=None, op0=ALU.add,
                    )
            for mt in range(g * 4, (g + 1) * 4):
                ps = ps_pool.tile([P, D], F32, tag="av", bufs=4, name="psv")
                for k in range(DC):
                    nc.tensor.matmul(
                        ps,
                        lhsT=xkvT[:, mt, k, :],
                        rhs=wkv_sb[:, k, D:2 * D],
                        start=(k == 0), stop=False,
                    )
                nc.tensor.matmul(
                    ps, lhsT=ones1p, rhs=vb_sb, start=False, stop=True,
                )
                nc.vector.tensor_copy(
                    out=vaug[:, mt, :, 0:HD],
                    in_=ps.rearrange("p (h d) -> p h d", h=H),
                )
            for m in range(DC):
                ps = ps_pool.tile([P, IC], F32, tag="av", bufs=4, name="psq")
                for k in range(DC):
                    nc.tensor.matmul(
                        ps,
                        lhsT=wq_sb[:, k, m * P:(m + 1) * P],
                        rhs=xqT[:, g * 4:(g + 1) * 4, k, :],
                        start=(k == 0), stop=(k == DC - 1),
                    )
                if g < 3:
                    nc.scalar.activation(
                        out=qT[:, m, g * IC:(g + 1) * IC], in_=ps,
                        func=ACTF.Identity, bias=qb_sb[:, m:m + 1],
                    )
                else:
                    nc.vector.tensor_scalar(
                        out=qT[:, m, g * IC:(g + 1) * IC], in0=ps,
                        scalar1=qb_sb[:, m:m + 1], scalar2=None, op0=ALU.add,
                    )

        # ---- phase C: attention ----
        # Head PAIRS: the two heads of chunk hp live at partitions 0-63 /
        # 64-127, so their K=64 sim matmuls run CONCURRENTLY via row-group
        # tiling. Pass (2 query chunks) is the OUTER loop so the first half
        # of the output projection can overlap the second pass. PSUM: one
        # 4-bank sim tile (both sides) + av x4 = 8 banks; a single
        # [128,2048] Exp per j-tile covers both sides.
        def _outproj_group(g):
            # K=128 output projection (head pairs packed) for one group of 4
            # token tiles; a dense PE burst that also keeps HAM warm
            os_big = bigp.tile([P, 4, D], BF16, tag="big", name="os_big")
            for j in range(4):
                it = g * 4 + j
                ps = ps_pool.tile([P, D], F32, tag="av", bufs=4, name="pso")
                for hp in range(H // 2):
                    nc.tensor.matmul(
                        ps,
                        lhsT=aout[:, hp, it * P:(it + 1) * P],
                        rhs=wout_sb[:, hp, :],
                        start=(hp == 0), stop=(hp == H // 2 - 1),
                    )
                nc.vector.tensor_copy(out=os_big[:, j, :], in_=ps)
            nc.sync.dma_start(out=outs[g][:, :, :], in_=os_big)

        for pas in range(2):
            ics = (2 * pas, 2 * pas + 1)
            for hp in range(H // 2):
                avs = {}
                for side in range(2):
                    for ic in ics:
                        avs[side, ic] = ps_pool.tile(
                            [P, IC], F32, tag="av", bufs=4,
                            name=f"av{side}{ic}",
                        )
                for jt in range(NT):
                    sims = []
                    for side in range(2):
                        sims.append(ps_pool.tile(
                            [P, 2 * IC], F32, tag=f"sim{side}", bufs=1,
                            name=f"sim{side}",
                        ))
                    # interleave sides: adjacent matmuls hit different PE
                    # row groups and overlap in the array
                    for k, ic in enumerate(ics):
                        for side in range(2):
                            hb = side * HD
                            nc.tensor.matmul(
                                sims[side][:, k * IC:(k + 1) * IC],
                                lhsT=kT[hb:hb + HD, hp, jt * P:(jt + 1) * P],
                                rhs=qT[hb:hb + HD, hp,
                                       ic * IC:(ic + 1) * IC],
                                start=True, stop=True,
                            )
                    for side in range(2):
                        ex = expp.tile([P, 2 * IC], BF16, tag="exp",
                                       bufs=6, name="ex")
                        nc.scalar.activation(out=ex, in_=sims[side],
                                             func=ACTF.Exp)
                        for k, ic in enumerate(ics):
                            nc.tensor.matmul(
                                avs[side, ic][0:HD + 1, :],
                                lhsT=vaug[:, jt, 2 * hp + side, :],
                                rhs=ex[:, k * IC:(k + 1) * IC],
                                start=(jt == 0), stop=(jt == NT - 1),
                            )
                # normalization: one fp32 copy frees each AV bank; batch the
                # 4 denominators into one DVE reciprocal via a DMA gather
                avsbs = {}
                den_dram = dramp.tile([4, IC], F32, name="den")
                for j, (side, ic) in enumerate(
                        (s, i) for s in range(2) for i in ics):
                    avsb = avsbp.tile([HD + 1, IC], F32, tag="avsb",
                                      bufs=8, name="avsb")
                    nc.vector.tensor_copy(out=avsb,
                                          in_=avs[side, ic][0:HD + 1, :])
                    avsbs[side, ic] = avsb
                    nc.sync.dma_start(out=den_dram[j:j + 1, :],
                                      in_=avsb[HD:HD + 1, :])
                den4 = smallp.tile([4, IC], F32, tag="den4", name="den4")
                nc.sync.dma_start(out=den4, in_=den_dram[:, :])
                rec4 = smallp.tile([4, IC], F32, tag="rec4", name="rec4")
                nc.vector.reciprocal(out=rec4, in_=den4)
                rd4 = dramp.tile([4, IC], F32, name="rd4")
                nc.sync.dma_start(out=rd4, in_=rec4)
                for j, (side, ic) in enumerate(
                        (s, i) for s in range(2) for i in ics):
                    bcast = smallp.tile([HD, IC], F32, tag="bcast",
                                        name="bcast")
                    row = rd4[j:j + 1, :]
                    row_b = bass.AP(
                        tensor=row.tensor, offset=row.offset,
                        ap=[[0, HD]] + list(row.ap)[1:],
                    )
                    nc.sync.dma_start(out=bcast, in_=row_b)
                    hb = side * HD
                    nc.gpsimd.tensor_tensor(
                        out=aout[hb:hb + HD, hp, ic * IC:(ic + 1) * IC],
                        in0=avsbs[side, ic][0:HD, :], in1=bcast,
                        op=ALU.mult,
                    )
                if pas == 1 and hp < 2:
                    # pass-0 queries' output projection rides the hp
                    # transition of pass 1 (dense PE burst, no idle dip)
                    _outproj_group(hp)
        for g in (2, 3):
            _outproj_group(g)

def _get_nc():
    global _NC_CACHE
    if _NC_CACHE is None:
        _NC_CACHE = _build_program()
    return _NC_CACHE


def kernel(i, t, g_i, b_i, g_t, b_t, w_qkv_i, w_qkv_t, w_out_i, w_out_t):
    global LAST_EXEC_NS
    nc = _get_nc()

    bf = ml_dtypes.bfloat16
    i = np.ascontiguousarray(np.asarray(i, np.float32)).astype(bf)
    t = np.ascontiguousarray(np.asarray(t, np.float32)).astype(bf)
    w_qkv_i = np.asarray(w_qkv_i, np.float32)
    w_qkv_t = np.asarray(w_qkv_t, np.float32)
    g_i = np.asarray(g_i, np.float32)
    b_i = np.asarray(b_i, np.float32)
    g_t = np.asarray(g_t, np.float32)
    b_t = np.asarray(b_t, np.float32)

    # fold gamma into the weight rows; beta contributes beta@W as a bias.
    # 0.125 = head_dim**-0.5 folded into the q projection/bias.
    def prep(w_qkv, g, b):
        wq_f = (g[:, None] * w_qkv[:, :D] * 0.125).astype(bf)
        wkv_f = np.ascontiguousarray(g[:, None] * w_qkv[:, D:]).astype(bf)
        qb_f = (b @ w_qkv[:, :D] * 0.125).astype(np.float32)
        kb_f = (b @ w_qkv[:, D:2 * D]).astype(np.float32)
        vb_f = (b @ w_qkv[:, 2 * D:]).astype(bf)
        return wq_f, wkv_f, qb_f, kb_f, vb_f

    wq_i, wkv_i, qb_i, kb_i, vb_i = prep(w_qkv_i, g_i, b_i)
    wq_t, wkv_t, qb_t, kb_t, vb_t = prep(w_qkv_t, g_t, b_t)
    wo_i = np.asarray(w_out_i, np.float32).astype(bf)
    wo_t = np.asarray(w_out_t, np.float32).astype(bf)

    in_maps = []
    for c in range(8):
        b, d = c // 2, c % 2
        if d == 0:  # t -> i: queries from t, keys/values from i
            m = dict(xq=np.ascontiguousarray(t[b]),
                     xkv=np.ascontiguousarray(i[b]),
                     wq=wq_t, wkv=wkv_i, wout=wo_i,
                     qb=qb_t, kb=kb_i, vb=vb_i)
        else:       # i -> t
            m = dict(xq=np.ascontiguousarray(i[b]),
                     xkv=np.ascontiguousarray(t[b]),
                     wq=wq_i, wkv=wkv_t, wout=wo_t,
                     qb=qb_i, kb=kb_t, vb=vb_t)
        in_maps.append(m)

    res = run_bass_kernel_spmd(nc, in_maps, list(range(8)))
    LAST_EXEC_NS = res.exec_time_ns

    out = np.empty((4, 2 * N, D), np.float32)
    for c in range(8):
        b, d = c // 2, c % 2
        for g in range(NT // 4):
            blk = np.asarray(res.results[c][f"out{g}"]).astype(np.float32)
            for j in range(4):
                it = g * 4 + j
                out[b, d * N + it * P:d * N + (it + 1) * P, :] = blk[:, j, :]
    return out


# revision 19
# speedup vs baseline: 1.1755x; 1.1755x over previous
"""Trainium2 Bass kernel: dual cross-attention block (nn_CA_36670430773307).

Full-input contract: kernel(**inputs) takes the complete unsharded tensors and
returns the complete (4, 4096, 512) output.

Sharding: 8 cores = batch(4) x direction(2). Each core computes one full
cross-attention direction (t->i or i->t) for one batch element:
    xq_ln = LN(xq);  xkv_ln = LN(xkv)
    q = xq_ln @ w_q * 0.125          (0.125 and gamma folded into w_q on host)
    k = xkv_ln @ w_k ; v = xkv_ln @ w_v
    out = softmax(q @ k^T) @ v @ w_out          (per head, 8 heads)
gamma is folded into the projection weights on the host; beta contributes
beta@W, added as a per-partition bias for feature-major q/k and via a K=1
ones-matmul for token-major v.

Engine assignment keeps the ACT (scalar) engine exclusively on softmax Exp —
the kernel's hard floor (33.5M exp elements ~ 300us at 1 elem/cycle/lane):
  - LN stats: bn_stats/bn_aggr on DVE; batched sqrt(var+eps) on ACT ([P,4]
    per 512-token group) and one batched DVE reciprocal
  - LN apply: tensor_scalar on the GPSIMD engine (SBUF->SBUF)
  - transpose to d-major x^T: dma_start_transpose (XBAR, SBUF->SBUF, bit
    exact) into token-tile-major xT2 [P, NT, DC, 128] whose per-partition
    destination blocks are contiguous -- no PE transposes, no PSUM traffic
  - projections per 512-token group, PSUM->SBUF copies on DVE (bias folded)
  - attention per (head-pair, 2-query-chunk pass): sim^T = k_h @ q_h^T (the
    two heads of a pair live on partition halves so their K=64 matmuls run
    concurrently via PE row groups), Exp over [128,1024] PSUM -> bf16, AV
    matmuls accumulate [65,512] with a ones column producing the softmax
    denominator in row 64
  - normalization: one fp32 copy [65,512] frees the AV PSUM bank (so the
    next pass's AV matmuls never stall and the PE stays warm/un-throttled);
    denominator rows are DMA-gathered into [4,512], ONE batched DVE
    reciprocal per pass, DMA broadcast back, multiply on GPSIMD
  - output projection: head pairs stacked on 128 partitions -> K=128
    matmuls accumulated in one PSUM bank -> bf16 out (host upcasts)
No max-subtraction in softmax: logits are ~N(0, 0.2), exp is safe.

Walrus allows 1 sync wait per instruction; _legalize_waits splits extras
onto same-engine EventSemaphore instructions. scalar_tensor_tensor (opcode
0x9d) is rejected by this NRT at NEFF load -- avoided.
"""

import numpy as np
import ml_dtypes

import concourse.bass as bass
import concourse.mybir as mybir
import concourse.tile as tile
from concourse.bass_utils import run_bass_kernel_spmd

N = 2048            # tokens per stream
D = 512             # model dim
H = 8               # heads
HD = 64             # head dim
P = 128             # SBUF partitions
NT = N // P         # 16 token tiles
DC = D // P         # 4 model-dim chunks
IC = 512            # attention query chunk == PSUM bank free size fp32
NIC = N // IC       # 4 query chunks
LN_EPS = 1e-5

F32 = mybir.dt.float32
BF16 = mybir.dt.bfloat16
ALU = mybir.AluOpType
ACTF = mybir.ActivationFunctionType

LAST_EXEC_NS = None
_NC_CACHE = None


def _legalize_waits(js):
    """Walrus encodes ONE sync wait per instruction (the ISA EVENTS slot).
    Tile's wait assignment can attach several. Split the surplus onto
    preceding same-engine EventSemaphore instructions."""
    n_split = 0
    for f in js["functions"]:
        for b in f["blocks"]:
            out = []
            for ins in b["instructions"]:
                si = ins.get("sync_info") or {}
                ow = si.get("on_wait") or []
                if len(ow) > 1:
                    for k, w in enumerate(ow[:-1]):
                        out.append({
                            "debug": ins.get("debug"),
                            "engine": ins["engine"],
                            "ins": [], "outs": [],
                            "name": f"{ins['name']}_w{k}",
                            "opcode": "EventSemaphore",
                            "sync_info": {"on_update": [], "on_wait": [w]},
                        })
                        n_split += 1
                    si = dict(si)
                    si["on_wait"] = [ow[-1]]
                    ins = dict(ins)
                    ins["sync_info"] = si
                out.append(ins)
            b["instructions"] = out
    return n_split


def _build_program():
    nc = bass.Bass()

    xq = nc.declare_dram_parameter("xq", [N, D], BF16, isOutput=False)
    xkv = nc.declare_dram_parameter("xkv", [N, D], BF16, isOutput=False)
    wq = nc.declare_dram_parameter("wq", [D, D], BF16, isOutput=False)
    wkv = nc.declare_dram_parameter("wkv", [D, 2 * D], BF16, isOutput=False)
    wout = nc.declare_dram_parameter("wout", [D, D], BF16, isOutput=False)
    qb = nc.declare_dram_parameter("qb", [D], F32, isOutput=False)
    kb = nc.declare_dram_parameter("kb", [D], F32, isOutput=False)
    vb = nc.declare_dram_parameter("vb", [D], BF16, isOutput=False)
    outs = [
        nc.declare_dram_parameter(f"out{g}", [P, 4, D], BF16, isOutput=True)
        for g in range(NT // 4)
    ]

    with tile.TileContext(nc) as tc:
        _body(tc, xq, xkv, wq, wkv, wout, qb, kb, vb, outs)

    import json
    js = json.loads(nc.to_json_bytes())
    _legalize_waits(js)
    legalized = json.dumps(js).encode()
    nc.to_json_bytes = lambda: legalized
    return nc


def _ln_group(tc, lns, lnxs, src_big, g, eps_sb):
    """LN for 4 token tiles of one stream. Stats on DVE, batched sqrt on
    ACT, batched reciprocal on DVE, apply on GPSIMD. Returns bf16 tiles."""
    nc = tc.nc
    mvg = lns.tile([P, 4, 2], F32, tag="mv", name="mvg")
    for kk in range(4):
        it = g * 4 + kk
        st = lns.tile([P, 6], F32, tag="st", name="st")
        nc.vector.bn_stats(out=st, in_=src_big[:, it, :])
        nc.vector.bn_aggr(out=mvg[:, kk, :], in_=st)
    sdg = lns.tile([P, 4], F32, tag="sd", name="sdg")
    nc.scalar.activation(out=sdg, in_=mvg[:, :, 1], func=ACTF.Sqrt,
                         bias=eps_sb)
    invg = lns.tile([P, 4], F32, tag="inv", name="invg")
    nc.vector.reciprocal(out=invg, in_=sdg)
    nmig = lns.tile([P, 4], F32, tag="nmi", name="nmig")
    nc.vector.tensor_tensor(out=nmig, in0=mvg[:, :, 0], in1=invg,
                            op=ALU.mult)
    nc.vector.tensor_scalar(out=nmig, in0=nmig, scalar1=-1.0,
                            scalar2=None, op0=ALU.mult)
    xss = []
    for kk in range(4):
        it = g * 4 + kk
        xs = lnxs.tile([P, D], BF16, name="xs")
        nc.gpsimd.tensor_scalar(
            out=xs, in0=src_big[:, it, :],
            scalar1=invg[:, kk:kk + 1], scalar2=nmig[:, kk:kk + 1],
            op0=ALU.mult, op1=ALU.add,
        )
        xss.append(xs)
    return xss


def _body(tc, xq, xkv, wq, wkv, wout, qb, kb, vb, outs):
    nc = tc.nc

    with (
        tc.tile_pool(name="persist", bufs=1) as pers,
        tc.tile_pool(name="lns", bufs=8) as lns,
        tc.tile_pool(name="lnxs", bufs=10) as lnxs,
        tc.tile_pool(name="lnx", bufs=2) as lnx,
        tc.tile_pool(name="expp", bufs=2) as expp,
        tc.tile_pool(name="smallp", bufs=3) as smallp,
        tc.tile_pool(name="avsbp", bufs=8) as avsbp,
        tc.tile_pool(name="bigp", bufs=2) as bigp,
        tc.tile_pool(name="dramp", bufs=8, space="DRAM") as dramp,
        tc.tile_pool(name="ps_pool", bufs=2, space="PSUM") as ps_pool,
    ):
        # ---- persistent tiles ----
        eps_sb = pers.tile([P, 1], F32, name="eps_sb")
        nc.vector.memset(eps_sb, LN_EPS)
        ones1p = pers.tile([1, P], BF16, name="ones1p")
        nc.vector.memset(ones1p, 1.0)

        # weights + biases on the HW DGE (sync) ring; x loads on the gpsimd
        # SW ring (8 DMAs, one per token group) so SW lanes never wrap
        wq_sb = pers.tile([P, DC, D], BF16, name="wq_sb")
        nc.sync.dma_start(out=wq_sb, in_=wq.rearrange("(c p) f -> p c f", p=P))
        wkv_sb = pers.tile([P, DC, 2 * D], BF16, name="wkv_sb")
        nc.sync.dma_start(out=wkv_sb, in_=wkv.rearrange("(c p) f -> p c f", p=P))
        # w_out rows stacked per head PAIR: [128, H//2, D]
        wout_sb = pers.tile([P, H // 2, D], BF16, name="wout_sb")
        nc.sync.dma_start(out=wout_sb, in_=wout.rearrange("(c p) f -> p c f", p=P))
        qb_sb = pers.tile([P, DC], F32, name="qb_sb")
        nc.sync.dma_start(out=qb_sb, in_=qb.rearrange("(c p) -> p c", p=P))
        kb_sb = pers.tile([P, DC], F32, name="kb_sb")
        nc.sync.dma_start(out=kb_sb, in_=kb.rearrange("(c p) -> p c", p=P))
        vb_sb = pers.tile([1, D], BF16, name="vb_sb")
        nc.sync.dma_start(out=vb_sb, in_=vb.rearrange("(o d) -> o d", o=1))

        # x^T in token-tile-major layout: xT2[p, it, c, f] = x[it*128+f,
        # c*128+p]; each dma_start_transpose writes one contiguous
        # [128, DC*128] per-partition block
        xqT = bigp.tile([P, NT, DC, P], BF16, tag="big", name="xqT")
        xkvT = bigp.tile([P, NT, DC, P], BF16, tag="big", name="xkvT")
        qT = pers.tile([P, DC, N], BF16, name="qT")   # q^T feature-major
        kT = pers.tile([P, DC, N], BF16, name="kT")   # head h: chunk h//2,
        #                                               partitions (h%2)*64
        vaug = pers.tile([P, NT, H, HD + 1], BF16, name="vaug")
        nc.vector.memset(vaug[:, :, :, HD:HD + 1], 1.0)
        # normalized attention output, head pairs stacked on partitions:
        # head 2*hp+side lives at partitions side*64..(side+1)*64
        aout = pers.tile([P, H // 2, N], BF16, name="aout")

        xqbig = lnx.tile([P, NT, D], BF16, tag="xbig", name="xqbig")
        xkvbig = lnx.tile([P, NT, D], BF16, tag="xbig", name="xkvbig")
        xq_r = xq.rearrange("(t p) d -> p t d", p=P)
        xkv_r = xkv.rearrange("(t p) d -> p t d", p=P)

        # ---- phases A+B pipelined per 512-token group ----
        for g in range(NT // 4):
            nc.gpsimd.dma_start(out=xqbig[:, g * 4:(g + 1) * 4, :],
                                in_=xq_r[:, g * 4:(g + 1) * 4, :])
            nc.gpsimd.dma_start(out=xkvbig[:, g * 4:(g + 1) * 4, :],
                                in_=xkv_r[:, g * 4:(g + 1) * 4, :])
            for xbig, xT in ((xqbig, xqT), (xkvbig, xkvT)):
                xss = _ln_group(tc, lns, lnxs, xbig, g, eps_sb)
                for kk in range(4):
                    it = g * 4 + kk
                    nc.sync.dma_start_transpose(xT[:, it, :, :], xss[kk])

            # projections for this token group: k and v first (attention
            # consumes them per j-tile, so the tail group must land early),
            # q last. Copies for groups 0-2 ride the idle ACT engine; group
            # 3's go on DVE so the exp queue never waits on them.
            for m in range(DC):
                ps = ps_pool.tile([P, IC], F32, tag="av", bufs=4, name="psk")
                for k in range(DC):
                    nc.tensor.matmul(
                        ps,
                        lhsT=wkv_sb[:, k, m * P:(m + 1) * P],
                        rhs=xkvT[:, g * 4:(g + 1) * 4, k, :],
                        start=(k == 0), stop=(k == DC - 1),
                    )
                if g < 3:
                    nc.scalar.activation(
                        out=kT[:, m, g * IC:(g + 1) * IC], in_=ps,
                        func=ACTF.Identity, bias=kb_sb[:, m:m + 1],
                    )
                else:
                    nc.vector.tensor_scalar(
                        out=kT[:, m, g * IC:(g + 1) * IC], in0=ps,
                        scalar1=kb_sb[:, m:m + 1], scalar2=None, op0=ALU.add,
                    )
            for mt in range(g * 4, (g + 1) * 4):
                ps = ps_pool.tile([P, D], F32, tag="av", bufs=4, name="psv")
                for k in range(DC):
                    nc.tensor.matmul(
                        ps,
                        lhsT=xkvT[:, mt, k, :],
                        rhs=wkv_sb[:, k, D:2 * D],
                        start=(k == 0), stop=False,
                    )
                nc.tensor.matmul(
                    ps, lhsT=ones1p, rhs=vb_sb, start=False, stop=True,
                )
                nc.vector.tensor_copy(
                    out=vaug[:, mt, :, 0:HD],
                    in_=ps.rearrange("p (h d) -> p h d", h=H),
                )
            for m in range(DC):
                ps = ps_pool.tile([P, IC], F32, tag="av", bufs=4, name="psq")
                for k in range(DC):
                    nc.tensor.matmul(
                        ps,
                        lhsT=wq_sb[:, k, m * P:(m + 1) * P],
                        rhs=xqT[:, g * 4:(g + 1) * 4, k, :],
                        start=(k == 0), stop=(k == DC - 1),
                    )
                if g < 3:
                    nc.scalar.activation(
                        out=qT[:, m, g * IC:(g + 1) * IC], in_=ps,
                        func=ACTF.Identity, bias=qb_sb[:, m:m + 1],
                    )
                else:
                    nc.vector.tensor_scalar(
                        out=qT[:, m, g * IC:(g + 1) * IC], in0=ps,
                        scalar1=qb_sb[:, m:m + 1], scalar2=None, op0=ALU.add,
                    )

        # ---- phase C: attention ----
        # Head PAIRS: the two heads of chunk hp live at partitions 0-63 /
        # 64-127, so their K=64 sim matmuls run CONCURRENTLY via row-group
        # tiling. Pass (2 query chunks) is the OUTER loop so the first half
        # of the output projection can overlap the second pass. PSUM: one
        # 4-bank sim tile (both sides) + av x4 = 8 banks; a single
        # [128,2048] Exp per j-tile covers both sides.
        def _outproj_group(g):
            # K=128 output projection (head pairs packed) for one group of 4
            # token tiles; a dense PE burst that also keeps HAM warm
            os_big = bigp.tile([P, 4, D], BF16, tag="big", name="os_big")
            for j in range(4):
                it = g * 4 + j
                ps = ps_pool.tile([P, D], F32, tag="av", bufs=4, name="pso")
                for hp in range(H // 2):
                    nc.tensor.matmul(
                        ps,
                        lhsT=aout[:, hp, it * P:(it + 1) * P],
                        rhs=wout_sb[:, hp, :],
                        start=(hp == 0), stop=(hp == H // 2 - 1),
                    )
                nc.vector.tensor_copy(out=os_big[:, j, :], in_=ps)
            nc.sync.dma_start(out=outs[g][:, :, :], in_=os_big)

        for pas in range(2):
            ics = (2 * pas, 2 * pas + 1)
            for hp in range(H // 2):
                avs = {}
                for side in range(2):
                    for ic in ics:
                        avs[side, ic] = ps_pool.tile(
                            [P, IC], F32, tag="av", bufs=4,
                            name=f"av{side}{ic}",
                        )
                for jt in range(NT):
                    sims = []
                    for side in range(2):
                        sims.append(ps_pool.tile(
                            [P, 2 * IC], F32, tag=f"sim{side}", bufs=1,
                            name=f"sim{side}",
                        ))
                    # interleave sides: adjacent matmuls hit different PE
                    # row groups and overlap in the array
                    for k, ic in enumerate(ics):
                        for side in range(2):
                            hb = side * HD
                            nc.tensor.matmul(
                                sims[side][:, k * IC:(k + 1) * IC],
                                lhsT=kT[hb:hb + HD, hp, jt * P:(jt + 1) * P],
                                rhs=qT[hb:hb + HD, hp,
                                       ic * IC:(ic + 1) * IC],
                                start=True, stop=True,
                            )
                    for side in range(2):
                        ex = expp.tile([P, 2 * IC], BF16, tag="exp",
                                       bufs=4, name="ex")
                        nc.scalar.activation(out=ex, in_=sims[side],
                                             func=ACTF.Exp)
                        for k, ic in enumerate(ics):
                            nc.tensor.matmul(
                                avs[side, ic][0:HD + 1, :],
                                lhsT=vaug[:, jt, 2 * hp + side, :],
                                rhs=ex[:, k * IC:(k + 1) * IC],
                                start=(jt == 0), stop=(jt == NT - 1),
                            )
                # normalization: one fp32 copy frees each AV bank; batch the
                # 4 denominators into one DVE reciprocal via a DMA gather
                avsbs = {}
                den_dram = dramp.tile([4, IC], F32, name="den")
                for j, (side, ic) in enumerate(
                        (s, i) for s in range(2) for i in ics):
                    avsb = avsbp.tile([HD + 1, IC], F32, tag="avsb",
                                      bufs=8, name="avsb")
                    nc.vector.tensor_copy(out=avsb,
                                          in_=avs[side, ic][0:HD + 1, :])
                    avsbs[side, ic] = avsb
                    nc.sync.dma_start(out=den_dram[j:j + 1, :],
                                      in_=avsb[HD:HD + 1, :])
                den4 = smallp.tile([4, IC], F32, tag="den4", name="den4")
                nc.sync.dma_start(out=den4, in_=den_dram[:, :])
                rec4 = smallp.tile([4, IC], F32, tag="rec4", name="rec4")
                nc.vector.reciprocal(out=rec4, in_=den4)
                rd4 = dramp.tile([4, IC], F32, name="rd4")
                nc.sync.dma_start(out=rd4, in_=rec4)
                for j, (side, ic) in enumerate(
                        (s, i) for s in range(2) for i in ics):
                    bcast = smallp.tile([HD, IC], F32, tag="bcast",
                                        name="bcast")
                    row = rd4[j:j + 1, :]
                    row_b = bass.AP(
                        tensor=row.tensor, offset=row.offset,
                        ap=[[0, HD]] + list(row.ap)[1:],
                    )
                    nc.sync.dma_start(out=bcast, in_=row_b)
                    hb = side * HD
                    nc.gpsimd.tensor_tensor(
                        out=aout[hb:hb + HD, hp, ic * IC:(ic + 1) * IC],
                        in0=avsbs[side, ic][0:HD, :], in1=bcast,
                        op=ALU.mult,
                    )
                if pas == 1 and hp < 2:
                    # pass-0 queries' output projection rides the hp
                    # transition of pass 1 (dense PE burst, no idle dip)
                    _outproj_group(hp)
        for g in (2, 3):
            _outproj_group(g)

def _get_nc():
    global _NC_CACHE
    if _NC_CACHE is None:
        _NC_CACHE = _build_program()
    return _NC_CACHE


def kernel(i, t, g_i, b_i, g_t, b_t, w_qkv_i, w_qkv_t, w_out_i, w_out_t):
    global LAST_EXEC_NS
    nc = _get_nc()

    bf = ml_dtypes.bfloat16
    i = np.ascontiguousarray(np.asarray(i, np.float32)).astype(bf)
    t = np.ascontiguousarray(np.asarray(t, np.float32)).astype(bf)
    w_qkv_i = np.asarray(w_qkv_i, np.float32)
    w_qkv_t = np.asarray(w_qkv_t, np.float32)
    g_i = np.asarray(g_i, np.float32)
    b_i = np.asarray(b_i, np.float32)
    g_t = np.asarray(g_t, np.float32)
    b_t = np.asarray(b_t, np.float32)

    # fold gamma into the weight rows; beta contributes beta@W as a bias.
    # 0.125 = head_dim**-0.5 folded into the q projection/bias.
    def prep(w_qkv, g, b):
        wq_f = (g[:, None] * w_qkv[:, :D] * 0.125).astype(bf)
        wkv_f = np.ascontiguousarray(g[:, None] * w_qkv[:, D:]).astype(bf)
        qb_f = (b @ w_qkv[:, :D] * 0.125).astype(np.float32)
        kb_f = (b @ w_qkv[:, D:2 * D]).astype(np.float32)
        vb_f = (b @ w_qkv[:, 2 * D:]).astype(bf)
        return wq_f, wkv_f, qb_f, kb_f, vb_f

    wq_i, wkv_i, qb_i, kb_i, vb_i = prep(w_qkv_i, g_i, b_i)
    wq_t, wkv_t, qb_t, kb_t, vb_t = prep(w_qkv_t, g_t, b_t)
    wo_i = np.asarray(w_out_i, np.float32).astype(bf)
    wo_t = np.asarray(w_out_t, np.float32).astype(bf)

    in_maps = []
    for c in range(8):
        b, d = c // 2, c % 2
        if d == 0:  # t -> i: queries from t, keys/values from i
            m = dict(xq=np.ascontiguousarray(t[b]),
                     xkv=np.ascontiguousarray(i[b]),
                     wq=wq_t, wkv=wkv_i, wout=wo_i,
                     qb=qb_t, kb=kb_i, vb=vb_i)
        else:       # i -> t
            m = dict(xq=np.ascontiguousarray(i[b]),
                     xkv=np.ascontiguousarray(t[b]),
                     wq=wq_i, wkv=wkv_t, wout=wo_t,
                     qb=qb_i, kb=kb_t, vb=vb_t)
        in_maps.append(m)

    res = run_bass_kernel_spmd(nc, in_maps, list(range(8)))
    LAST_EXEC_NS = res.exec_time_ns

    out = np.empty((4, 2 * N, D), np.float32)
    for c in range(8):
        b, d = c // 2, c % 2
        for g in range(NT // 4):
            blk = np.asarray(res.results[c][f"out{g}"]).astype(np.float32)
            for j in range(4):
                it = g * 4 + j
                out[b, d * N + it * P:d * N + (it + 1) * P, :] = blk[:, j, :]
    return out
